# revision 32
# baseline (speedup 1.0000x reference)
"""Multi-head attention on 8 Trainium2 NeuronCores (tensor-parallel over heads).

B=4, S=2048, D=1024, H=16 heads of DK=64. Each core owns 2 heads (a
128-channel slice of the QKV projections). x is pre-transposed on the
host to [B, D, S] so the device reads contiguous rows (no DMA transpose).

Per core, per batch b (all matmul operands bf16, fp32 PSUM accumulate):
  QT   = Wq_c^T xT + bq    [128, S]   (both heads stacked on partitions)
  KT   = Wk_c^T xT         [128, S]   (bk cancels in softmax -- dropped)
  V    = xT^T Wv_c         [S, 128] stored per-head [128, 16, 64]
  attention per (q-pair, 512-wide sub), accumulating over k-chunks kc:
    sc[128, 2, 512] psum = K_h Q_h^T for h0, h1 -- the two C=64 matmuls
       are issued adjacently so they run concurrently in the PE array
       (row tiling at partitions 0/64); sc double-buffered so ACT and PE
       ping-pong without stalls
    ex = exp(sc/8)  one ACT instr, N=1024 (amortizes the 352-cyc overhead)
    av[128, 512] psum += [V0^T ex0 ; V1^T ex1]  (col-tiled pair, M=64
       each at col positions 0/64 -- ctx^T lands in O-projection layout)
    se psum += ones^T ex   (2 col-tiled M=1 matmuls; rows 0/32 for sub0,
       64/96 for sub1, one se bank per q-pair)
  rse = 1/se (one DVE reciprocal per q-pair), broadcast via DRAM bounce,
  ctx = av * rse on DVE, out partial = ctx^T Wo_c  [S, D] bf16.
Host sums the 8 cores' partials and adds bo + bv@Wo (bv commutes through
softmax since the attention weights sum to 1).
"""

import numpy as np

B, S, D, H, DK = 4, 2048, 1024, 16, 64
NCORES = 8
CS = D // NCORES   # 128 channels (2 heads) per core
NDC = D // 128     # 8 d-chunks
NKC = S // 128     # 16 k-chunks
NST = S // 512     # 4 s-tiles
NQP = S // 1024    # 2 q-pairs

TRACE = False
LAST_RESULTS = None
_CACHE = {}


def _interleave(main, fill, start_frac=0.2):
    """Spread fill units evenly between main units (order preserved).
    No fill before start_frac of main has been emitted: the engines run
    in static order, so a fill unit whose inputs aren't ready yet would
    stall them."""
    out = []
    fi = 0
    n0 = int(len(main) * start_frac)
    for i, u in enumerate(main):
        out.append(u)
        if i < n0:
            continue
        want = (i - n0 + 1) * len(fill) // max(1, len(main) - n0)
        while fi < want:
            out.append(fill[fi])
            fi += 1
    out.extend(fill[fi:])
    return out


def _build():
    import concourse.bass as bass  # noqa: F401
    import concourse.mybir as mybir
    import concourse.tile as tile
    from concourse import bacc

    fp32 = mybir.dt.float32
    bf16 = mybir.dt.bfloat16
    AF = mybir.ActivationFunctionType

    nc = bacc.Bacc(None, target_bir_lowering=False)
    xt_d = nc.declare_dram_parameter("xt", [B, D, S], bf16, isOutput=False)
    out_d = nc.declare_dram_parameter("out", [B, S, D], bf16, isOutput=True)
    wq_d = nc.declare_dram_parameter("wq", [128, NDC, CS], bf16, isOutput=False)
    wk_d = nc.declare_dram_parameter("wk", [128, NDC, CS], bf16, isOutput=False)
    wv_d = nc.declare_dram_parameter("wv", [128, NDC, CS], bf16, isOutput=False)
    wo_d = nc.declare_dram_parameter("wo", [CS, D], bf16, isOutput=False)
    bq_d = nc.declare_dram_parameter("bq", [CS], fp32, isOutput=False)
    # batch 0's first 512 s-columns, host-packed into contiguous 8KB rows
    # so the first projection chains start ~15us before the full xT lands
    xh_d = nc.declare_dram_parameter("xh", [128, NDC, 512], bf16, isOutput=False)

    with tile.TileContext(nc) as tc:
        with (
            tc.tile_pool(name="consts", bufs=1) as consts,
            tc.tile_pool(name="xt", bufs=2) as xt_pool,
            tc.tile_pool(name="qk", bufs=2) as qk_pool,
            tc.tile_pool(name="vp", bufs=2) as v_pool,
            tc.tile_pool(name="exq", bufs=6) as ex_pool,
            tc.tile_pool(name="ctx", bufs=2) as ctx_pool,
            tc.tile_pool(name="avs", bufs=4) as avs_pool,
            tc.tile_pool(name="rec", bufs=2) as rec_pool,
            tc.tile_pool(name="rb", bufs=8) as rb_pool,
            tc.tile_pool(name="outp", bufs=4) as out_pool,
            tc.tile_pool(name="drp", bufs=8, space="DRAM") as dram_pool,
            tc.tile_pool(name="pssc", bufs=2, space="PSUM") as ps_sc,
            tc.tile_pool(name="psav", bufs=2, space="PSUM") as ps_av,
            tc.tile_pool(name="psse", bufs=1, space="PSUM") as ps_se,
            tc.tile_pool(name="pspj", bufs=1, space="PSUM") as ps_pj,
        ):
            wq_t = consts.tile([128, NDC, CS], bf16, tag="wq")
            wk_t = consts.tile([128, NDC, CS], bf16, tag="wk")
            wv_t = consts.tile([128, NDC, CS], bf16, tag="wv")
            wo_t = consts.tile([128, D], bf16, tag="wo")
            bq_t = consts.tile([128, 1], fp32, tag="bq")
            ones_t = consts.tile([128, 1], bf16, tag="ones")

            def load_consts():
                # wq/wk/wv arrive host-permuted as [128, NDC, CS] so these
                # are contiguous row DMAs
                nc.sync.dma_start(wq_t[:], wq_d[:])
                nc.sync.dma_start(wk_t[:], wk_d[:])
                nc.sync.dma_start(wv_t[:], wv_d[:])
                nc.sync.dma_start(wo_t[:], wo_d[:])
                nc.sync.dma_start(bq_t[:], bq_d[:].rearrange("(p o) -> p o", o=1))
                nc.gpsimd.memset(ones_t[:], 1.0)

            state = {}

            def A_xdma(bi, b):
                xT = xt_pool.tile([128, NDC, S], bf16, tag="xT")
                state[bi] = dict(xT=xT)
                xr = xt_d[b].rearrange("(c p) M -> p c M", p=128)
                return [
                    (lambda cch=cch: nc.sync.dma_start(
                        xT[:, cch, :], xr[:, cch]))
                    for cch in range(NDC)
                ]

            def A_units(bi, xh=None):
                st_ = state[bi]
                xT = st_["xT"]
                # group-0 units read the early-landing head tensor instead
                xof = {}
                if xh is not None:
                    xof[0] = xh
                QT = qk_pool.tile([128, S], bf16, tag="QT")
                KT = qk_pool.tile([128, S], bf16, tag="KT")
                v0 = v_pool.tile([128, NKC, DK], bf16, tag="v0")
                v1 = v_pool.tile([128, NKC, DK], bf16, tag="v1")
                st_.update(QT=QT, KT=KT, v0=v0, v1=v1)
                # All projections run as F=128 accumulation chains packing
                # four [128,128] quarters into one psum bank -- this keeps
                # LDWEIGHTS pipelined (measured ~61ns/MM vs ~300ns at F=512)
                units = []
                for w_t, dst, is_q in ((wq_t, QT, True), (wk_t, KT, False)):
                    for g in range(4):
                        qcarry = {}
                        for j in range(4):
                            def u_p(g=g, j=j, w_t=w_t, dst=dst, is_q=is_q,
                                    qcarry=qcarry):
                                if j == 0:
                                    pq = ps_pj.tile([128, 512], fp32, tag="pj")
                                    qcarry["pq"] = pq
                                pq = qcarry["pq"]
                                sb = g * 4 + j
                                qsl = slice(j * 128, (j + 1) * 128)
                                xs = xof.get(g, xT)
                                xo = g * 512 if xs is xT else 0
                                for cch in range(NDC):
                                    nc.tensor.matmul(
                                        pq[:, qsl], w_t[:, cch, :],
                                        xs[:, cch,
                                           xo + j * 128 : xo + (j + 1) * 128],
                                        start=(cch == 0), stop=(cch == NDC - 1),
                                        skip_group_check=True,
                                    )
                                if j == 3:
                                    sl = slice(g * 512, (g + 1) * 512)
                                    if is_q:
                                        nc.vector.tensor_scalar_add(
                                            dst[:, sl], pq[:], bq_t[:]
                                        )
                                    else:
                                        nc.vector.tensor_copy(dst[:, sl], pq[:])
                            units.append(u_p)

                for g in range(4):  # groups of 4 s-blocks
                    vcarry = {}
                    for j in range(4):
                        def u_v(g=g, j=j, vcarry=vcarry):
                            if j == 0:
                                pv = ps_pj.tile([128, 512], fp32, tag="pj")
                                vcarry["pv"] = pv
                            pv = vcarry["pv"]
                            sb = g * 4 + j
                            qsl = slice(j * 128, (j + 1) * 128)
                            xs = xof.get(g, xT)
                            xo = g * 512 if xs is xT else 0
                            for cch in range(NDC):
                                nc.tensor.matmul(
                                    pv[:, qsl],
                                    xs[:, cch,
                                       xo + j * 128 : xo + (j + 1) * 128],
                                    wv_t[:, cch, :],
                                    start=(cch == 0), stop=(cch == NDC - 1),
                                    skip_group_check=True,
                                )
                            if j == 3:
                                pvv = pv[:].rearrange("p (j c) -> p j c", j=4)
                                nc.vector.tensor_copy(
                                    v0[:, g * 4 : (g + 1) * 4, :],
                                    pvv[:, :, 0:DK],
                                )
                                nc.vector.tensor_copy(
                                    v1[:, g * 4 : (g + 1) * 4, :],
                                    pvv[:, :, DK:CS],
                                )
                        units.append(u_v)
                return units

            def B_units(bi):
                """Attention for batch bi: 2 sections (one per q-pair).
                Both 512-subs of the q-pair are processed per k-chunk so
                each stationary (K_h, V_h) serves two matmuls and the 4
                sumexp matmuls run as one 4-way col-tiled pass."""
                st_ = state[bi]
                QT, KT, v0, v1 = st_["QT"], st_["KT"], st_["v0"], st_["v1"]
                ctxs = []
                sections = []
                for qp in range(NQP):
                    q0 = qp * 1024
                    ctx = ctx_pool.tile([128, 1024], bf16, tag="ctx")
                    ctxs.append(ctx)
                    carry = {}
                    units = []

                    def u_start(carry=carry):
                        av0 = ps_av.tile([128, 512], fp32, tag="av")
                        av1 = ps_av.tile([128, 512], fp32, tag="av")
                        se = ps_se.tile([128, 512], fp32, tag="se")
                        nc.vector.memset(se[:], 1.0)
                        carry.update(av=(av0, av1), se=se, ex={})
                    units.append(u_start)

                    def u_sc(kc, q0=q0, carry=carry):
                        ksl = slice(kc * 128, (kc + 1) * 128)
                        exs = []
                        for sub in range(2):
                            qsl = slice(q0 + sub * 512, q0 + (sub + 1) * 512)
                            sc = ps_sc.tile([128, 2, 512], fp32, tag="sc")
                            # h0/h1 adjacent -> concurrent row tiles (0/64);
                            # K stationaries persist across the two subs
                            nc.tensor.matmul(
                                sc[:, 0, :], KT[0:DK, ksl], QT[0:DK, qsl],
                                start=True, stop=True,
                            )
                            nc.tensor.matmul(
                                sc[:, 1, :], KT[DK:CS, ksl], QT[DK:CS, qsl],
                                start=True, stop=True,
                            )
                            ex = ex_pool.tile([128, 2, 512], bf16, tag="ex")
                            nc.scalar.activation(
                                ex[:], sc[:], AF.Exp, scale=0.125
                            )
                            exs.append(ex)
                        carry["ex"][kc] = exs

                    def u_av(kc, carry=carry):
                        ex0, ex1 = carry["ex"].pop(kc)
                        av0, av1 = carry["av"]
                        se = carry["se"]
                        first, last = kc == 0, kc == NKC - 1
                        # col-tiled pairs: h0 -> rows 0-63, h1 -> 64-127;
                        # V stationaries persist across the two subs
                        for ex, av in ((ex0, av0), (ex1, av1)):
                            nc.tensor.matmul(
                                av[0:DK, :], v0[:, kc, :], ex[:, 0, :],
                                start=first, stop=last, skip_group_check=True,
                            )
                            nc.tensor.matmul(
                                av[DK:CS, :], v1[:, kc, :], ex[:, 1, :],
                                start=first, stop=last, skip_group_check=True,
                            )
                        # sumexp: 4-way col-tiled pass, rows (sub, h) ->
                        # 0:(s0,h0) 32:(s0,h1) 64:(s1,h0) 96:(s1,h1)
                        for qi, ex in enumerate((ex0, ex0, ex1, ex1)):
                            p = 32 * qi
                            nc.tensor.matmul(
                                se[p : p + 1, :], ones_t[:], ex[:, qi % 2, :],
                                start=first, stop=last,
                                skip_group_check=True,
                                tile_position=(0, p),
                            )

                    for kc in range(NKC):
                        def u_kc(kc=kc, u_sc=u_sc, u_av=u_av):
                            u_sc(kc)
                            if kc > 0:
                                u_av(kc - 1)
                            if kc == NKC - 1:
                                u_av(kc)
                        units.append(u_kc)

                    def u_end(carry=carry):
                        # drain the av/se psum banks quickly; the rest of
                        # the normalize chain (u_norm) is emitted a few
                        # units into the next section so the 3 DVE ops +
                        # DRAM bounce don't block the pipeline here
                        av = carry["av"]
                        avss = []
                        for s in range(2):
                            avs = avs_pool.tile([128, 512], fp32, tag="avs")
                            nc.vector.tensor_copy(avs[:], av[s][:])
                            avss.append(avs)
                        rse = rec_pool.tile([128, 512], fp32, tag="rse")
                        nc.vector.reciprocal_approx_fast(rse[:], carry["se"][:])
                        carry.update(avss=avss, rse=rse)
                    units.append(u_end)

                    def u_norm(carry=carry, ctx=ctx):
                        rse = carry["rse"]
                        for s in range(2):
                            # rb rows 0-63 <- 1/se(h0), 64-127 <- 1/se(h1)
                            rb = rb_pool.tile([128, 512], fp32, tag="rb")
                            for h in range(2):
                                dr = dram_pool.tile([1, 512], fp32, tag="dr")
                                nc.sync.dma_start(
                                    dr[:],
                                    rse[64 * s + 32 * h : 64 * s + 32 * h + 1, :],
                                )
                                nc.sync.dma_start(
                                    rb[h * DK : (h + 1) * DK, :],
                                    dr[:].partition_broadcast(DK),
                                )
                            ssl = slice(s * 512, (s + 1) * 512)
                            nc.vector.tensor_mul(
                                ctx[:, ssl], carry["avss"][s][:], rb[:]
                            )
                    sections.append((units, u_norm))
                st_["ctx"] = ctxs
                return sections

            def C_units(bi, b, qp):
                ctx = state[bi]["ctx"][qp]
                units = []
                for j in range(8):
                    def u_o(j=j):
                        sb = qp * 8 + j
                        lsl = slice(j * 128, (j + 1) * 128)
                        ot = out_pool.tile([128, D], bf16, tag="ot")
                        # borrow a tile from the sc rotation (2 banks =
                        # both halves) instead of contending for the
                        # single Q/K/V projection bank
                        po = ps_sc.tile([128, 2, 512], fp32, tag="sc")
                        for half in range(2):
                            for oc in range(4):
                                o0 = half * 512 + oc * 128
                                nc.tensor.matmul(
                                    po[:, half, oc * 128 : (oc + 1) * 128],
                                    ctx[:, lsl], wo_t[:, o0 : o0 + 128],
                                    start=True, stop=True,
                                    skip_group_check=True,
                                )
                            nc.vector.tensor_copy(
                                ot[:, half * 512 : (half + 1) * 512],
                                po[:, half, :],
                            )
                        nc.sync.dma_start(
                            out_d[b, sb * 128 : (sb + 1) * 128, :], ot[:]
                        )
                    units.append(u_o)
                return units

            # ---- software pipeline over batches ----
            xh_t = consts.tile([128, NDC, 512], bf16, tag="xh")
            nc.sync.dma_start(xh_t[:], xh_d[:])
            for u in A_xdma(0, 0):
                u()
            load_consts()
            # batch 0 ramp: emit only the projection prefix attention
            # needs immediately (Q g0-1, K g0, V g0); the rest interleaves
            # into batch 0's own attention, ordered by first use.
            a0 = A_units(0, xh=xh_t)
            a_cur = []
            for u in a0[0:4] + a0[16:20] + a0[32:36] + a0[4:8]:
                u()
            a0_rest = []
            for g in range(1, 4):
                a0_rest += a0[16 + 4 * g : 20 + 4 * g]  # K g
                a0_rest += a0[32 + 4 * g : 36 + 4 * g]  # V g
            a0_rest += a0[8:16]  # Q g2, g3
            c_prev = []   # C units of (bi-1, qp1)
            norm_prev = []  # deferred normalize of (bi-1, qp1)
            for bi in range(B):
                for u in a_cur:
                    u()
                (sec0, norm0), (sec1, norm1) = B_units(bi)
                if bi + 1 < B:
                    for u in A_xdma(bi + 1, bi + 1):
                        u()
                    a_next = A_units(bi + 1)
                else:
                    a_next = []
                half = len(a_next) // 2
                # Fill order: batch-0 leftovers / deferred normalize first
                # (their consumers sit later in the list), then next
                # batch's projections, then the freshest out-projection.
                for u in _interleave(
                    sec0, a0_rest + norm_prev + a_next[:half] + c_prev, 0.08
                ):
                    u()
                a0_rest = []
                c_q0 = C_units(bi, bi, 0)
                for u in _interleave(
                    sec1, [norm0] + a_next[half:] + c_q0, 0.08
                ):
                    u()
                c_prev = C_units(bi, bi, 1)
                norm_prev = [norm1]
                a_cur = []
            for u in norm_prev:
                u()
            for u in c_prev:
                u()

    nc.compile()
    return nc


def _get_nc():
    if "nc" not in _CACHE:
        _CACHE["nc"] = _build()
    return _CACHE["nc"]


def kernel(**inputs):
    global LAST_RESULTS
    import ml_dtypes
    from concourse.bass_utils import run_bass_kernel_spmd

    bf = ml_dtypes.bfloat16
    x = np.asarray(inputs["x"], dtype=np.float32)
    xt = np.ascontiguousarray(x.transpose(0, 2, 1)).astype(bf)  # [B, D, S]
    Wq = np.asarray(inputs["Wq"], dtype=np.float32).astype(bf)
    Wk = np.asarray(inputs["Wk"], dtype=np.float32).astype(bf)
    Wv = np.asarray(inputs["Wv"], dtype=np.float32).astype(bf)
    Wo = np.asarray(inputs["Wo"], dtype=np.float32).astype(bf)
    bq = np.asarray(inputs["bq"], dtype=np.float32)
    bv = np.asarray(inputs["bv"], dtype=np.float32)
    bo = np.asarray(inputs["bo"], dtype=np.float32)

    def permute_w(w):  # [D, CS] -> [128, NDC, CS] (partition-major chunks)
        return np.ascontiguousarray(
            w.reshape(NDC, 128, CS).transpose(1, 0, 2)
        )

    xh = np.ascontiguousarray(
        xt[0].reshape(NDC, 128, S)[:, :, :512].transpose(1, 0, 2)
    )
    nc = _get_nc()
    in_maps = []
    for c in range(NCORES):
        cs = slice(CS * c, CS * (c + 1))
        in_maps.append(
            {
                "xt": xt,
                "xh": xh,
                "wq": permute_w(Wq[:, cs]),
                "wk": permute_w(Wk[:, cs]),
                "wv": permute_w(Wv[:, cs]),
                "wo": np.ascontiguousarray(Wo[cs, :]),
                "bq": np.ascontiguousarray(bq[cs]),
            }
        )
    res = run_bass_kernel_spmd(
        nc, in_maps, core_ids=list(range(NCORES)), trace=TRACE
    )
    LAST_RESULTS = res
    acc = np.zeros((B, S, D), dtype=np.float64)
    for c in range(NCORES):
        acc += np.asarray(res.results[c]["out"], dtype=np.float64)
    # bk drops out of softmax; bv commutes through (sum of weights = 1)
    acc += bo + bv.astype(np.float64) @ np.asarray(
        inputs["Wo"], dtype=np.float64
    )
    return acc.astype(np.float32)
